# revision 24
# baseline (speedup 1.0000x reference)
"""DeepAR Trainium2 Bass kernel.

Strategy (hardcoded from spec nn_DeepAR_90374701843258):
  B=32, LIN=96, LOUT=24, N=256, E=32, H=64, T-1=119 steps, 8 cores.
  Data-parallel over B: 4 batch rows per core -> per-core batch BN=1024.

  Layout: "folded" tiles: partition p<64 = H-unit p of batch half 0
  (bn 0:512), p>=64 = H-unit p-64 of half 1 (bn 512:1024). The free dim
  is the within-half batch column. The 1024 batch is further split into
  two independent GROUPS (cols 0:256, 256:512 of each half) that pipeline
  against each other across engines.

  Algebra:
   - embedding + layer0 input proj collapse to rank-1: pre0 = x*w_eff + b_eff
     (w_eff = Wih0 @ embed_W); x and ones live in a pre-staged xall buffer
     (partitions 0/1/64/65, one column block per step) and enter the gate
     matmul as extra contraction rows.
   - block-diagonal stationary diag(Wx^T, Wx^T) [128,128] lets one matmul
     produce a folded gate tile (both halves) per group.
   - i,f gates: real Sigmoid on ACT; g: Tanh; o: tanh(o/2) via 0.5-prescaled
     weights, sigmoid recovered as 0.5*tanh+0.5 with a 4x-mode tensor_scalar.
   - cell update: u=si*tg, v=sf*c, c'=u+v, tc=tanh(c'), so=0.5*to2+0.5,
     h=so*tc  (all bf16 TT at 2x / TS at 4x DVE modes).
"""

import numpy as np

B, LIN, LOUT, N, E, H = 32, 96, 24, 256, 32, 64
T = LIN + LOUT
TS = T - 1            # 119
NCORES = 8
BL = B // NCORES      # 4
BN = BL * N           # 1024
HALF = 512
GW = 256              # group width (columns of folded tiles)
NCHUNK = BN // 128    # 8

_cache = {}


def _pack_weights(inp):
    """Host-side weight prep (tiny arrays)."""
    import ml_dtypes
    bf16 = ml_dtypes.bfloat16
    f32 = np.float32

    Wih0, Whh0 = inp["Wih0"].astype(f32), inp["Whh0"].astype(f32)
    Wih1, Whh1 = inp["Wih1"].astype(f32), inp["Whh1"].astype(f32)
    w_eff = (Wih0 @ inp["embed_W"].astype(f32))[:, 0]
    b_eff = Wih0 @ inp["embed_b"].astype(f32) + inp["bih0"] + inp["bhh0"]
    b1 = (inp["bih1"] + inp["bhh1"]).astype(f32)

    sc = np.ones(4 * H, f32)
    sc[3 * H:] = 0.5       # o-gate only

    def blockdiag(Wm):
        out = np.zeros((128, 4 * 128), f32)
        for X in range(4):
            wt = (Wm[X * H:(X + 1) * H].T * sc[X * H:(X + 1) * H][None, :])
            out[0:64, X * 128:X * 128 + 64] = wt
            out[64:128, X * 128 + 64:(X + 1) * 128] = wt
        return out

    WH0 = blockdiag(Whh0)
    WI1 = blockdiag(Wih1)
    WH1 = blockdiag(Whh1)

    WX0 = np.zeros((128, 4 * 128), f32)
    for X in range(4):
        we = w_eff[X * H:(X + 1) * H] * sc[X * H:(X + 1) * H]
        be = b_eff[X * H:(X + 1) * H] * sc[X * H:(X + 1) * H]
        WX0[0, X * 128 + 64:(X + 1) * 128] = we   # x half1 -> out parts 64:128
        WX0[1, X * 128 + 64:(X + 1) * 128] = be
        WX0[64, X * 128:X * 128 + 64] = we        # x half0 -> out parts 0:64
        WX0[65, X * 128:X * 128 + 64] = be

    B1T = np.zeros((128, 4 * 128), f32)
    for X in range(4):
        bb = b1[X * H:(X + 1) * H] * sc[X * H:(X + 1) * H]
        B1T[32, X * 128:X * 128 + 64] = bb
        B1T[32, X * 128 + 64:(X + 1) * 128] = bb

    HD = np.zeros((128, 4), f32)
    HD[0:64, 0] = inp["mu_W"].astype(f32)[0]
    HD[0:64, 1] = inp["sigma_W"].astype(f32)[0]
    HD[64:128, 2] = inp["mu_W"].astype(f32)[0]
    HD[64:128, 3] = inp["sigma_W"].astype(f32)[0]

    return {
        "WH0": WH0.astype(bf16), "WX0": WX0.astype(bf16),
        "WI1": WI1.astype(bf16), "WH1": WH1.astype(bf16),
        "B1T": B1T.astype(bf16), "HD": HD.astype(bf16),
        "ID": np.eye(128, dtype=f32).astype(bf16),
        "ONESBIG": np.ones((1, TS * HALF), f32).astype(bf16),
        "IDF": np.eye(128, dtype=f32),
        "mu_b": float(inp["mu_b"][0]), "sigma_b": float(inp["sigma_b"][0]),
    }


def _build(mu_b, sigma_b):
    """Build the per-core bass program (SPMD: identical on all cores)."""
    from contextlib import ExitStack
    import concourse.mybir as mybir
    import concourse.tile as tile
    from concourse import bacc

    dt = mybir.dt
    AF = mybir.ActivationFunctionType
    OP = mybir.AluOpType

    nc = bacc.Bacc()

    # ---- I/O ----------------------------------------------------------
    hist = nc.declare_dram_parameter("hist", [BL, LIN, N], dt.float32, isOutput=False)
    fut = nc.declare_dram_parameter("fut", [BL, LOUT, N], dt.float32, isOutput=False)
    hmask = nc.declare_dram_parameter("hmask", [BL, LIN, N], dt.float32, isOutput=False)
    fmask = nc.declare_dram_parameter("fmask", [BL, LOUT, N], dt.float32, isOutput=False)
    epsin = nc.declare_dram_parameter("epsin", [BL, TS, N], dt.float32, isOutput=False)
    wWH0 = nc.declare_dram_parameter("WH0", [128, 512], dt.bfloat16, isOutput=False)
    wWX0 = nc.declare_dram_parameter("WX0", [128, 512], dt.bfloat16, isOutput=False)
    wWI1 = nc.declare_dram_parameter("WI1", [128, 512], dt.bfloat16, isOutput=False)
    wWH1 = nc.declare_dram_parameter("WH1", [128, 512], dt.bfloat16, isOutput=False)
    wB1T = nc.declare_dram_parameter("B1T", [128, 512], dt.bfloat16, isOutput=False)
    wHD = nc.declare_dram_parameter("HD", [128, 4], dt.bfloat16, isOutput=False)
    wID = nc.declare_dram_parameter("ID", [128, 128], dt.bfloat16, isOutput=False)
    wONB = nc.declare_dram_parameter("ONESBIG", [1, TS * HALF], dt.bfloat16, isOutput=False)
    wIDF = nc.declare_dram_parameter("IDF", [128, 128], dt.float32, isOutput=False)

    o_preds = nc.declare_dram_parameter("preds", [BL, TS, N], dt.float32, isOutput=True)
    o_reals = nc.declare_dram_parameter("reals", [BL, TS, N], dt.float32, isOutput=True)
    o_mus = nc.declare_dram_parameter("musv", [BL, TS, N], dt.float32, isOutput=True)
    o_sigs = nc.declare_dram_parameter("sigmasv", [BL, TS, N], dt.float32, isOutput=True)
    o_mask = nc.declare_dram_parameter("maskv", [BL, TS, N], dt.float32, isOutput=True)

    musig_d = nc.dram_tensor("musig", [4, 128, HALF], dt.bfloat16)
    xs_d = nc.dram_tensor("xsd", [TS, BN], dt.bfloat16)

    with ExitStack() as ctx:
        tc = ctx.enter_context(tile.TileContext(nc))
        persist = ctx.enter_context(tc.tile_pool(name="persist", bufs=1))
        work = ctx.enter_context(tc.tile_pool(name="work", bufs=3))
        psl0 = ctx.enter_context(tc.tile_pool(name="psl0", bufs=1, space="PSUM"))
        psl1 = ctx.enter_context(tc.tile_pool(name="psl1", bufs=1, space="PSUM"))

        # ---- weights ------------------------------------------------------
        WH0 = persist.tile([128, 512], dt.bfloat16, tag="WH0")
        WX0 = persist.tile([128, 512], dt.bfloat16, tag="WX0")
        WI1 = persist.tile([128, 512], dt.bfloat16, tag="WI1")
        WH1 = persist.tile([128, 512], dt.bfloat16, tag="WH1")
        B1T = persist.tile([128, 512], dt.bfloat16, tag="B1T")
        HD = persist.tile([128, 4], dt.bfloat16, tag="HD")
        ID = persist.tile([128, 128], dt.bfloat16, tag="ID")
        IDF = persist.tile([128, 128], dt.float32, tag="IDF")
        for t_, d_ in [(WH0, wWH0), (WX0, wWX0), (WI1, wWI1), (WH1, wWH1),
                       (B1T, wB1T), (HD, wHD), (ID, wID), (IDF, wIDF)]:
            nc.sync.dma_start(out=t_[:], in_=d_[:])

        # ---- per-group state ----------------------------------------------
        def gtile(nm, shape, dtype=dt.bfloat16):
            return [persist.tile(shape, dtype, tag=f"{nm}{g}", name=f"{nm}{g}")
                    for g in range(2)]

        ht0 = gtile("ht0", [128, GW])
        ht1 = gtile("ht1", [128, GW])
        c0t = gtile("c0t", [128, GW])
        c1t = gtile("c1t", [128, GW])
        for g in range(2):
            for t_ in (ht0[g], ht1[g], c0t[g], c1t[g]):
                nc.vector.memset(t_, 0.0)
        tg0 = gtile("tg0", [128, 4 * GW])
        tg1 = gtile("tg1", [128, 4 * GW])
        ua0 = gtile("ua0", [128, GW])
        vb0 = gtile("vb0", [128, GW])
        ua1 = gtile("ua1", [128, GW])
        vb1 = gtile("vb1", [128, GW])
        tc0 = gtile("tc0", [128, GW])
        tc1 = gtile("tc1", [128, GW])
        so0 = gtile("so0", [128, GW])
        so1 = gtile("so1", [128, GW])
        ms8 = [[persist.tile([4, 8 * GW], dt.bfloat16, tag=f"ms8{g}{i}",
                             name=f"ms8{g}{i}") for i in range(2)]
               for g in range(2)]

        xall = persist.tile([128, TS * HALF], dt.bfloat16, tag="xall")
        nc.vector.memset(xall, 0.0)
        for p_ in (1, 32, 65):
            nc.sync.dma_start(out=xall[p_:p_ + 1, :], in_=wONB[:])

        xt = persist.tile([TS, BN], dt.bfloat16, tag="xt")

        full_c, mv_c, stdev_c, istd_c = [], [], [], []

        # ---- pre-pass: stats, normalize, transpose x ----------------------
        for c in range(NCHUNK):
            b_, n0 = c // 2, (c % 2) * 128
            raw = work.tile([T, 128], dt.float32, tag="raw")
            nc.sync.dma_start(out=raw[0:LIN, :], in_=hist[b_, :, n0:n0 + 128])
            nc.sync.dma_start(out=raw[LIN:T, :], in_=fut[b_, :, n0:n0 + 128])
            fpt = psl1.tile([128, T], dt.float32, tag="l1g0", name="fpt")
            nc.tensor.transpose(fpt, raw, IDF[0:T, 0:T])
            fc = persist.tile([128, T], dt.float32, tag=f"full{c}", name=f"full{c}")
            nc.vector.tensor_copy(fc, fpt)

            st6 = work.tile([128, 6], dt.float32, tag="st6")
            mv = persist.tile([128, 2], dt.float32, tag=f"mv{c}", name=f"mv{c}")
            nc.vector.bn_stats(out=st6, in_=fc[:, 0:LIN])
            nc.vector.bn_aggr(out=mv, in_=st6)
            veps = work.tile([128, 1], dt.float32, tag="veps")
            nc.vector.tensor_scalar(out=veps, in0=mv[:, 1:2], scalar1=1e-5,
                                    scalar2=None, op0=OP.add)
            y0 = work.tile([128, 1], dt.float32, tag="y0")
            nc.scalar.activation(y0, veps, AF.Sqrt)
            r0 = work.tile([128, 1], dt.float32, tag="r0")
            nc.vector.reciprocal(r0, y0)
            yy = work.tile([128, 1], dt.float32, tag="yy")
            nc.vector.tensor_tensor(out=yy, in0=y0, in1=y0, op=OP.mult)
            e_ = work.tile([128, 1], dt.float32, tag="e_")
            nc.vector.tensor_tensor(out=e_, in0=veps, in1=yy, op=OP.subtract)
            d_ = work.tile([128, 1], dt.float32, tag="d_")
            nc.vector.scalar_tensor_tensor(out=d_, in0=e_, scalar=0.5, in1=r0,
                                           op0=OP.mult, op1=OP.mult)
            sd = persist.tile([128, 1], dt.float32, tag=f"sd{c}", name=f"sd{c}")
            nc.vector.tensor_tensor(out=sd, in0=y0, in1=d_, op=OP.add)
            isd = persist.tile([128, 1], dt.float32, tag=f"isd{c}", name=f"isd{c}")
            nc.vector.reciprocal(isd, sd)
            full_c.append(fc); mv_c.append(mv); stdev_c.append(sd); istd_c.append(isd)

            xn = work.tile([128, TS], dt.bfloat16, tag="xn")
            nc.vector.tensor_scalar(out=xn, in0=fc[:, 0:TS], scalar1=mv[:, 0:1],
                                    scalar2=isd, op0=OP.subtract, op1=OP.mult)
            pt = psl0.tile([TS, 128], dt.bfloat16, tag="l0g0", name="pt")
            nc.tensor.transpose(pt, xn, ID)
            xtcol = (1 - c // 4) * HALF + (c % 4) * 128
            nc.vector.tensor_copy(xt[:, xtcol:xtcol + 128], pt)

        # stage xt -> DRAM -> xall partitions 0 / 64
        nc.sync.dma_start(out=xs_d[:], in_=xt[:])
        nc.sync.dma_start(
            out=xall[0:1, :].rearrange("p (t b) -> p t b", b=HALF),
            in_=xs_d[None, :, 0:HALF])
        nc.sync.dma_start(
            out=xall[64:65, :].rearrange("p (t b) -> p t b", b=HALF),
            in_=xs_d[None, :, HALF:BN])

        # ---- main loop ----------------------------------------------------
        GS = [slice(X * 128, (X + 1) * 128) for X in range(4)]

        def l0_matmuls(ps, g, ts_):
            for X in range(4):
                nc.tensor.matmul(ps[:, X * GW:(X + 1) * GW], lhsT=WH0[:, GS[X]],
                                 rhs=ht0[g][:, :], start=True, stop=False)
                nc.tensor.matmul(ps[:, X * GW:(X + 1) * GW], lhsT=WX0[0:66, GS[X]],
                                 rhs=xall[0:66, ts_:ts_ + GW],
                                 start=False, stop=True)

        def l1_matmuls(ps, g, ts_):
            for X in range(4):
                nc.tensor.matmul(ps[:, X * GW:(X + 1) * GW], lhsT=WI1[:, GS[X]],
                                 rhs=ht0[g][:, :], start=True, stop=False)
                nc.tensor.matmul(ps[:, X * GW:(X + 1) * GW], lhsT=WH1[:, GS[X]],
                                 rhs=ht1[g][:, :], start=False, stop=False)
                nc.tensor.matmul(ps[:, X * GW:(X + 1) * GW], lhsT=B1T[32:33, GS[X]],
                                 rhs=xall[32:33, ts_:ts_ + GW],
                                 start=False, stop=True, tile_position=(32, 0))

        def gate_acts(ps, tgb):
            nc.scalar.activation(tgb[:, 0:2 * GW], ps[:, 0:2 * GW], AF.Sigmoid)
            nc.scalar.activation(tgb[:, 2 * GW:4 * GW], ps[:, 2 * GW:4 * GW],
                                 AF.Tanh)

        def cell_update(tgb, ct, ub, vb, tcb, sob, htile):
            si = tgb[:, 0:GW]
            sf = tgb[:, GW:2 * GW]
            tg_ = tgb[:, 2 * GW:3 * GW]
            to2 = tgb[:, 3 * GW:4 * GW]
            nc.vector.tensor_tensor(out=ub, in0=si, in1=tg_, op=OP.mult)
            nc.vector.tensor_tensor(out=vb, in0=sf, in1=ct, op=OP.mult)
            nc.vector.tensor_tensor(out=ct, in0=ub, in1=vb, op=OP.add)
            nc.scalar.activation(tcb, ct, AF.Tanh)
            nc.vector.tensor_scalar(out=sob, in0=to2, scalar1=0.5, scalar2=0.5,
                                    op0=OP.mult, op1=OP.add)
            nc.vector.tensor_tensor(out=htile, in0=sob, in1=tcb, op=OP.mult)

        def heads_mm(t, g):
            hps = psl0.tile([4, GW], dt.float32, tag=f"l0g{g}", name="hps")
            nc.tensor.matmul(hps, lhsT=HD[:, 0:4], rhs=ht1[g][:, :],
                             start=True, stop=True)
            ring = ms8[g][(t // 8) % 2]
            nc.vector.tensor_copy(ring[:, (t % 8) * GW:(t % 8 + 1) * GW], hps)
            if t % 8 == 7 or t == TS - 1:
                k0 = t - (t % 8)
                nw = t - k0 + 1
                nc.sync.dma_start(
                    out=musig_d[:, k0:t + 1, g * GW:(g + 1) * GW],
                    in_=ring[:, 0:nw * GW].rearrange("h (s b) -> h s b", b=GW))

        for t in range(TS):
            l0ps = {}
            for g in range(2):
                ts_ = t * HALF + g * GW
                l0ps[g] = psl0.tile([128, 4 * GW], dt.float32, tag=f"l0g{g}",
                                    name="l0ps")
                l0_matmuls(l0ps[g], g, ts_)
                if t > 0:
                    heads_mm(t - 1, g)
            for g in range(2):
                gate_acts(l0ps[g], tg0[g])
                cell_update(tg0[g], c0t[g], ua0[g], vb0[g], tc0[g], so0[g],
                            ht0[g])
            l1ps = {}
            for g in range(2):
                ts_ = t * HALF + g * GW
                l1ps[g] = psl1.tile([128, 4 * GW], dt.float32, tag=f"l1g{g}",
                                    name="l1ps")
                l1_matmuls(l1ps[g], g, ts_)
            for g in range(2):
                gate_acts(l1ps[g], tg1[g])
                cell_update(tg1[g], c1t[g], ua1[g], vb1[g], tc1[g], so1[g],
                            ht1[g])
        for g in range(2):
            heads_mm(TS - 1, g)

        # ---- post-pass ----------------------------------------------------
        c_sigb = persist.tile([128, 1], dt.float32, tag="c_sigb")
        nc.vector.memset(c_sigb, sigma_b)
        c_neg1 = persist.tile([128, 1], dt.float32, tag="c_neg1")
        nc.vector.memset(c_neg1, -1.0)

        for c in range(NCHUNK):
            b_, n0 = c // 2, (c % 2) * 128
            fc, mv, sd, isd = full_c[c], mv_c[c], stdev_c[c], istd_c[c]

            mu_tf = work.tile([128, 128], dt.bfloat16, tag="mu_tf")
            sg_tf = work.tile([128, 128], dt.bfloat16, tag="sg_tf")
            nc.sync.dma_start_transpose(
                out=mu_tf,
                in_=musig_d[0 + 2 * (c // 4), :, (c % 4) * 128:(c % 4 + 1) * 128])
            nc.sync.dma_start_transpose(
                out=sg_tf,
                in_=musig_d[1 + 2 * (c // 4), :, (c % 4) * 128:(c % 4 + 1) * 128])
            mu_t = mu_tf[:, 0:TS]
            sg_t = sg_tf[:, 0:TS]

            eps_c = work.tile([128, TS], dt.float32, tag="eps_c")
            nc.sync.dma_start(out=eps_c,
                              in_=epsin[b_, :, n0:n0 + 128].rearrange("t n -> n t"))
            mk = work.tile([128, TS], dt.float32, tag="mk")
            nc.sync.dma_start(out=mk[:, 0:LIN - 1],
                              in_=hmask[b_, 1:LIN, n0:n0 + 128].rearrange("t n -> n t"))
            nc.sync.dma_start(out=mk[:, LIN - 1:TS],
                              in_=fmask[b_, :, n0:n0 + 128].rearrange("t n -> n t"))

            # sigma = softplus(sg + sigma_b) + 1e-6
            ab_ = work.tile([128, TS], dt.float32, tag="ab_")
            nc.scalar.activation(ab_, sg_t, AF.Abs, bias=c_sigb)
            ex_ = work.tile([128, TS], dt.float32, tag="ex_")
            nc.scalar.activation(ex_, ab_, AF.Exp, scale=c_neg1)
            ln_ = work.tile([128, TS], dt.float32, tag="ln_")
            nc.scalar.activation(ln_, ex_, AF.Ln, bias=1.0)
            rl_ = work.tile([128, TS], dt.float32, tag="rl_")
            nc.vector.tensor_scalar(out=rl_, in0=sg_t, scalar1=sigma_b,
                                    scalar2=0.0, op0=OP.add, op1=OP.max)
            sig = work.tile([128, TS], dt.float32, tag="sig")
            nc.vector.scalar_tensor_tensor(out=sig, in0=ln_, scalar=1e-6, in1=rl_,
                                           op0=OP.add, op1=OP.add)

            # preds = ((mu+mu_b) + sigma*eps)*stdev + means, masked
            m1 = work.tile([128, TS], dt.float32, tag="m1")
            nc.vector.tensor_tensor(out=m1, in0=sig, in1=eps_c, op=OP.mult)
            m2 = work.tile([128, TS], dt.float32, tag="m2")
            nc.vector.scalar_tensor_tensor(out=m2, in0=mu_t, scalar=mu_b, in1=m1,
                                           op0=OP.add, op1=OP.add)
            m3 = work.tile([128, TS], dt.float32, tag="m3")
            nc.vector.tensor_scalar(out=m3, in0=m2, scalar1=sd, scalar2=mv[:, 0:1],
                                    op0=OP.mult, op1=OP.add)
            pr = work.tile([128, TS], dt.float32, tag="pr")
            nc.vector.tensor_tensor(out=pr, in0=m3, in1=mk, op=OP.mult)

            rr = work.tile([128, TS], dt.float32, tag="rr")
            nc.vector.tensor_tensor(out=rr, in0=fc[:, 1:T], in1=mk, op=OP.mult)

            u1 = work.tile([128, TS], dt.float32, tag="u1")
            nc.vector.tensor_scalar(out=u1, in0=mu_t, scalar1=mu_b, scalar2=None,
                                    op0=OP.add)
            u2 = work.tile([128, TS], dt.float32, tag="u2")
            nc.vector.tensor_scalar(out=u2, in0=u1, scalar1=sd, scalar2=mv[:, 0:1],
                                    op0=OP.mult, op1=OP.add)

            v1 = work.tile([128, TS], dt.float32, tag="v1")
            nc.vector.tensor_scalar(out=v1, in0=sig, scalar1=sd, scalar2=mv[:, 0:1],
                                    op0=OP.mult, op1=OP.add)

            for src_t, odram in ((pr, o_preds), (rr, o_reals), (u2, o_mus),
                                 (v1, o_sigs), (mk, o_mask)):
                tps = psl0.tile([TS, 128], dt.float32, tag="l0g0", name="tps")
                nc.tensor.transpose(tps, src_t, IDF)
                osb = work.tile([TS, 128], dt.float32, tag="osb", bufs=4)
                nc.vector.tensor_copy(osb, tps)
                nc.sync.dma_start(out=odram[b_, :, n0:n0 + 128], in_=osb)

    nc.finalize()
    return nc


def kernel(**inputs):
    import os
    from concourse.bass_utils import run_bass_kernel_spmd

    f32 = np.float32
    packs = _pack_weights(inputs)

    key = "nc"
    if key not in _cache:
        _cache[key] = _build(packs["mu_b"], packs["sigma_b"])
    nc = _cache[key]

    hist = np.ascontiguousarray(np.asarray(inputs["history_data"], f32)[..., 0])
    fut = np.ascontiguousarray(np.asarray(inputs["future_data"], f32)[..., 0])
    hm = np.ascontiguousarray(np.asarray(inputs["history_mask"], f32))
    fm = np.ascontiguousarray(np.asarray(inputs["future_mask"], f32))
    eps = np.ascontiguousarray(np.asarray(inputs["eps"], f32)[..., 0])

    in_maps = []
    for c in range(NCORES):
        b0, b1 = c * BL, (c + 1) * BL
        m = {
            "hist": hist[b0:b1], "fut": fut[b0:b1],
            "hmask": hm[b0:b1], "fmask": fm[b0:b1], "epsin": eps[b0:b1],
        }
        for k in ("WH0", "WX0", "WI1", "WH1", "B1T", "HD", "ID", "ONESBIG",
                  "IDF"):
            m[k] = packs[k]
        in_maps.append(m)

    kres = run_bass_kernel_spmd(nc, in_maps, list(range(NCORES)),
                                trace=bool(os.environ.get("KERNEL_TRACE")))
    _cache["last"] = kres
    res = kres.results

    def gather(name):
        full = np.concatenate([res[c][name] for c in range(NCORES)], axis=0)
        return full.reshape(B, TS, N, 1).astype(f32)

    return (gather("preds"), gather("reals"), gather("musv"),
            gather("sigmasv"), gather("maskv"))


# revision 25
# speedup vs baseline: 1.0572x; 1.0572x over previous
"""DeepAR Trainium2 Bass kernel.

Strategy (hardcoded from spec nn_DeepAR_90374701843258):
  B=32, LIN=96, LOUT=24, N=256, E=32, H=64, T-1=119 steps, 8 cores.
  Data-parallel over B: 4 batch rows per core -> per-core batch BN=1024.

  Layout: "folded" tiles: partition p<64 = H-unit p of batch half 0
  (bn 0:512), p>=64 = H-unit p-64 of half 1 (bn 512:1024). The free dim
  is the within-half batch column. The 1024 batch is further split into
  two independent GROUPS (cols 0:256, 256:512 of each half) that pipeline
  against each other across engines.

  Algebra:
   - embedding + layer0 input proj collapse to rank-1: pre0 = x*w_eff + b_eff
     (w_eff = Wih0 @ embed_W); x and ones live in a pre-staged xall buffer
     (partitions 0/1/64/65, one column block per step) and enter the gate
     matmul as extra contraction rows.
   - block-diagonal stationary diag(Wx^T, Wx^T) [128,128] lets one matmul
     produce a folded gate tile (both halves) per group.
   - i,f gates: real Sigmoid on ACT; g: Tanh; o: tanh(o/2) via 0.5-prescaled
     weights, sigmoid recovered as 0.5*tanh+0.5 with a 4x-mode tensor_scalar.
   - cell update: u=si*tg, v=sf*c, c'=u+v, tc=tanh(c'), so=0.5*to2+0.5,
     h=so*tc  (all bf16 TT at 2x / TS at 4x DVE modes).
"""

import numpy as np

B, LIN, LOUT, N, E, H = 32, 96, 24, 256, 32, 64
T = LIN + LOUT
TS = T - 1            # 119
NCORES = 8
BL = B // NCORES      # 4
BN = BL * N           # 1024
HALF = 512
GW = 256              # group width (columns of folded tiles)
NCHUNK = BN // 128    # 8

_cache = {}


def _pack_weights(inp):
    """Host-side weight prep (tiny arrays)."""
    import ml_dtypes
    bf16 = ml_dtypes.bfloat16
    f32 = np.float32

    Wih0, Whh0 = inp["Wih0"].astype(f32), inp["Whh0"].astype(f32)
    Wih1, Whh1 = inp["Wih1"].astype(f32), inp["Whh1"].astype(f32)
    w_eff = (Wih0 @ inp["embed_W"].astype(f32))[:, 0]
    b_eff = Wih0 @ inp["embed_b"].astype(f32) + inp["bih0"] + inp["bhh0"]
    b1 = (inp["bih1"] + inp["bhh1"]).astype(f32)

    sc = np.ones(4 * H, f32)
    sc[3 * H:] = 0.5       # o-gate only

    def blockdiag(Wm):
        out = np.zeros((128, 4 * 128), f32)
        for X in range(4):
            wt = (Wm[X * H:(X + 1) * H].T * sc[X * H:(X + 1) * H][None, :])
            out[0:64, X * 128:X * 128 + 64] = wt
            out[64:128, X * 128 + 64:(X + 1) * 128] = wt
        return out

    WH0 = blockdiag(Whh0)
    WI1 = blockdiag(Wih1)
    WH1 = blockdiag(Whh1)

    WX0 = np.zeros((128, 4 * 128), f32)
    for X in range(4):
        we = w_eff[X * H:(X + 1) * H] * sc[X * H:(X + 1) * H]
        be = b_eff[X * H:(X + 1) * H] * sc[X * H:(X + 1) * H]
        WX0[0, X * 128 + 64:(X + 1) * 128] = we   # x half1 -> out parts 64:128
        WX0[1, X * 128 + 64:(X + 1) * 128] = be
        WX0[64, X * 128:X * 128 + 64] = we        # x half0 -> out parts 0:64
        WX0[65, X * 128:X * 128 + 64] = be

    B1T = np.zeros((128, 4 * 128), f32)
    for X in range(4):
        bb = b1[X * H:(X + 1) * H] * sc[X * H:(X + 1) * H]
        B1T[32, X * 128:X * 128 + 64] = bb
        B1T[32, X * 128 + 64:(X + 1) * 128] = bb

    HD = np.zeros((128, 4), f32)
    HD[0:64, 0] = inp["mu_W"].astype(f32)[0]
    HD[0:64, 1] = inp["sigma_W"].astype(f32)[0]
    HD[64:128, 2] = inp["mu_W"].astype(f32)[0]
    HD[64:128, 3] = inp["sigma_W"].astype(f32)[0]

    return {
        "WH0": WH0.astype(bf16), "WX0": WX0.astype(bf16),
        "WI1": WI1.astype(bf16), "WH1": WH1.astype(bf16),
        "B1T": B1T.astype(bf16), "HD": HD.astype(bf16),
        "ID": np.eye(128, dtype=f32).astype(bf16),
        "ONESBIG": np.ones((1, TS * HALF), f32).astype(bf16),
        "IDF": np.eye(128, dtype=f32),
        "mu_b": float(inp["mu_b"][0]), "sigma_b": float(inp["sigma_b"][0]),
    }


def _build(mu_b, sigma_b):
    """Build the per-core bass program (SPMD: identical on all cores)."""
    from contextlib import ExitStack
    import concourse.mybir as mybir
    import concourse.tile as tile
    from concourse import bacc

    dt = mybir.dt
    AF = mybir.ActivationFunctionType
    OP = mybir.AluOpType

    nc = bacc.Bacc()

    # ---- I/O ----------------------------------------------------------
    hist = nc.declare_dram_parameter("hist", [BL, LIN, N], dt.float32, isOutput=False)
    fut = nc.declare_dram_parameter("fut", [BL, LOUT, N], dt.float32, isOutput=False)
    hmask = nc.declare_dram_parameter("hmask", [BL, LIN, N], dt.float32, isOutput=False)
    fmask = nc.declare_dram_parameter("fmask", [BL, LOUT, N], dt.float32, isOutput=False)
    epsin = nc.declare_dram_parameter("epsin", [BL, TS, N], dt.float32, isOutput=False)
    wWH0 = nc.declare_dram_parameter("WH0", [128, 512], dt.bfloat16, isOutput=False)
    wWX0 = nc.declare_dram_parameter("WX0", [128, 512], dt.bfloat16, isOutput=False)
    wWI1 = nc.declare_dram_parameter("WI1", [128, 512], dt.bfloat16, isOutput=False)
    wWH1 = nc.declare_dram_parameter("WH1", [128, 512], dt.bfloat16, isOutput=False)
    wB1T = nc.declare_dram_parameter("B1T", [128, 512], dt.bfloat16, isOutput=False)
    wHD = nc.declare_dram_parameter("HD", [128, 4], dt.bfloat16, isOutput=False)
    wID = nc.declare_dram_parameter("ID", [128, 128], dt.bfloat16, isOutput=False)
    wONB = nc.declare_dram_parameter("ONESBIG", [1, TS * HALF], dt.bfloat16, isOutput=False)
    wIDF = nc.declare_dram_parameter("IDF", [128, 128], dt.float32, isOutput=False)

    o_preds = nc.declare_dram_parameter("preds", [BL, TS, N], dt.float32, isOutput=True)
    o_reals = nc.declare_dram_parameter("reals", [BL, TS, N], dt.float32, isOutput=True)
    o_mus = nc.declare_dram_parameter("musv", [BL, TS, N], dt.float32, isOutput=True)
    o_sigs = nc.declare_dram_parameter("sigmasv", [BL, TS, N], dt.float32, isOutput=True)
    o_mask = nc.declare_dram_parameter("maskv", [BL, TS, N], dt.float32, isOutput=True)

    musig_d = nc.dram_tensor("musig", [4, 128, HALF], dt.bfloat16)
    xs_d = nc.dram_tensor("xsd", [TS, BN], dt.bfloat16)

    with ExitStack() as ctx:
        tc = ctx.enter_context(tile.TileContext(nc))
        persist = ctx.enter_context(tc.tile_pool(name="persist", bufs=1))
        work = ctx.enter_context(tc.tile_pool(name="work", bufs=3))
        psl0 = ctx.enter_context(tc.tile_pool(name="psl0", bufs=1, space="PSUM"))
        psl1 = ctx.enter_context(tc.tile_pool(name="psl1", bufs=1, space="PSUM"))

        # ---- weights ------------------------------------------------------
        WH0 = persist.tile([128, 512], dt.bfloat16, tag="WH0")
        WX0 = persist.tile([128, 512], dt.bfloat16, tag="WX0")
        WI1 = persist.tile([128, 512], dt.bfloat16, tag="WI1")
        WH1 = persist.tile([128, 512], dt.bfloat16, tag="WH1")
        B1T = persist.tile([128, 512], dt.bfloat16, tag="B1T")
        HD = persist.tile([128, 4], dt.bfloat16, tag="HD")
        ID = persist.tile([128, 128], dt.bfloat16, tag="ID")
        IDF = persist.tile([128, 128], dt.float32, tag="IDF")
        for t_, d_ in [(WH0, wWH0), (WX0, wWX0), (WI1, wWI1), (WH1, wWH1),
                       (B1T, wB1T), (HD, wHD), (ID, wID), (IDF, wIDF)]:
            nc.sync.dma_start(out=t_[:], in_=d_[:])

        # ---- per-group state ----------------------------------------------
        def gtile(nm, shape, dtype=dt.bfloat16):
            return [persist.tile(shape, dtype, tag=f"{nm}{g}", name=f"{nm}{g}")
                    for g in range(2)]

        ht0 = gtile("ht0", [128, GW])
        ht1 = gtile("ht1", [128, GW])
        c0t = gtile("c0t", [128, GW])
        c1t = gtile("c1t", [128, GW])
        for g in range(2):
            for t_ in (ht0[g], ht1[g], c0t[g], c1t[g]):
                nc.vector.memset(t_, 0.0)
        tg0 = gtile("tg0", [128, 4 * GW])
        tg1 = gtile("tg1", [128, 4 * GW])
        ua0 = gtile("ua0", [128, GW])
        vb0 = gtile("vb0", [128, GW])
        ua1 = gtile("ua1", [128, GW])
        vb1 = gtile("vb1", [128, GW])
        tc0 = gtile("tc0", [128, GW])
        tc1 = gtile("tc1", [128, GW])
        so0 = gtile("so0", [128, GW])
        so1 = gtile("so1", [128, GW])
        ms8 = [[persist.tile([4, 8 * GW], dt.bfloat16, tag=f"ms8{g}{i}",
                             name=f"ms8{g}{i}") for i in range(2)]
               for g in range(2)]

        xall = persist.tile([128, TS * HALF], dt.bfloat16, tag="xall")
        nc.vector.memset(xall, 0.0)
        for p_ in (1, 32, 65):
            nc.sync.dma_start(out=xall[p_:p_ + 1, :], in_=wONB[:])

        xt = persist.tile([TS, BN], dt.bfloat16, tag="xt")

        full_c, mv_c, stdev_c, istd_c = [], [], [], []

        # ---- pre-pass: stats, normalize, transpose x ----------------------
        for c in range(NCHUNK):
            b_, n0 = c // 2, (c % 2) * 128
            raw = work.tile([T, 128], dt.float32, tag="raw")
            nc.sync.dma_start(out=raw[0:LIN, :], in_=hist[b_, :, n0:n0 + 128])
            nc.sync.dma_start(out=raw[LIN:T, :], in_=fut[b_, :, n0:n0 + 128])
            fpt = psl1.tile([128, T], dt.float32, tag="l1g0", name="fpt")
            nc.tensor.transpose(fpt, raw, IDF[0:T, 0:T])
            fc = persist.tile([128, T], dt.float32, tag=f"full{c}", name=f"full{c}")
            nc.vector.tensor_copy(fc, fpt)

            st6 = work.tile([128, 6], dt.float32, tag="st6")
            mv = persist.tile([128, 2], dt.float32, tag=f"mv{c}", name=f"mv{c}")
            nc.vector.bn_stats(out=st6, in_=fc[:, 0:LIN])
            nc.vector.bn_aggr(out=mv, in_=st6)
            veps = work.tile([128, 1], dt.float32, tag="veps")
            nc.vector.tensor_scalar(out=veps, in0=mv[:, 1:2], scalar1=1e-5,
                                    scalar2=None, op0=OP.add)
            y0 = work.tile([128, 1], dt.float32, tag="y0")
            nc.scalar.activation(y0, veps, AF.Sqrt)
            r0 = work.tile([128, 1], dt.float32, tag="r0")
            nc.vector.reciprocal(r0, y0)
            yy = work.tile([128, 1], dt.float32, tag="yy")
            nc.vector.tensor_tensor(out=yy, in0=y0, in1=y0, op=OP.mult)
            e_ = work.tile([128, 1], dt.float32, tag="e_")
            nc.vector.tensor_tensor(out=e_, in0=veps, in1=yy, op=OP.subtract)
            d_ = work.tile([128, 1], dt.float32, tag="d_")
            nc.vector.scalar_tensor_tensor(out=d_, in0=e_, scalar=0.5, in1=r0,
                                           op0=OP.mult, op1=OP.mult)
            sd = persist.tile([128, 1], dt.float32, tag=f"sd{c}", name=f"sd{c}")
            nc.vector.tensor_tensor(out=sd, in0=y0, in1=d_, op=OP.add)
            isd = persist.tile([128, 1], dt.float32, tag=f"isd{c}", name=f"isd{c}")
            nc.vector.reciprocal(isd, sd)
            full_c.append(fc); mv_c.append(mv); stdev_c.append(sd); istd_c.append(isd)

            xn = work.tile([128, TS], dt.bfloat16, tag="xn")
            nc.vector.tensor_scalar(out=xn, in0=fc[:, 0:TS], scalar1=mv[:, 0:1],
                                    scalar2=isd, op0=OP.subtract, op1=OP.mult)
            pt = psl0.tile([TS, 128], dt.bfloat16, tag="l0g0", name="pt")
            nc.tensor.transpose(pt, xn, ID)
            xtcol = (1 - c // 4) * HALF + (c % 4) * 128
            nc.vector.tensor_copy(xt[:, xtcol:xtcol + 128], pt)

        # stage xt -> DRAM -> xall partitions 0 / 64
        nc.sync.dma_start(out=xs_d[:], in_=xt[:])
        nc.sync.dma_start(
            out=xall[0:1, :].rearrange("p (t b) -> p t b", b=HALF),
            in_=xs_d[None, :, 0:HALF])
        nc.sync.dma_start(
            out=xall[64:65, :].rearrange("p (t b) -> p t b", b=HALF),
            in_=xs_d[None, :, HALF:BN])

        # ---- main loop ----------------------------------------------------
        GS = [slice(X * 128, (X + 1) * 128) for X in range(4)]

        def l0_matmuls(ps, g, ts_):
            for X in range(4):
                nc.tensor.matmul(ps[:, X * GW:(X + 1) * GW], lhsT=WH0[:, GS[X]],
                                 rhs=ht0[g][:, :], start=True, stop=False)
                nc.tensor.matmul(ps[:, X * GW:(X + 1) * GW], lhsT=WX0[0:66, GS[X]],
                                 rhs=xall[0:66, ts_:ts_ + GW],
                                 start=False, stop=True)

        def l1_matmuls(ps, g, ts_):
            for X in range(4):
                nc.tensor.matmul(ps[:, X * GW:(X + 1) * GW], lhsT=WI1[:, GS[X]],
                                 rhs=ht0[g][:, :], start=True, stop=False)
                nc.tensor.matmul(ps[:, X * GW:(X + 1) * GW], lhsT=WH1[:, GS[X]],
                                 rhs=ht1[g][:, :], start=False, stop=False)
                nc.tensor.matmul(ps[:, X * GW:(X + 1) * GW], lhsT=B1T[32:33, GS[X]],
                                 rhs=xall[32:33, ts_:ts_ + GW],
                                 start=False, stop=True, tile_position=(32, 0))

        def gate_acts(ps, tgb):
            nc.scalar.activation(tgb[:, 0:2 * GW], ps[:, 0:2 * GW], AF.Sigmoid)
            nc.scalar.activation(tgb[:, 2 * GW:4 * GW], ps[:, 2 * GW:4 * GW],
                                 AF.Tanh)

        def cell_update(tgb, ct, ub, vb, tcb, sob, htile):
            si = tgb[:, 0:GW]
            sf = tgb[:, GW:2 * GW]
            tg_ = tgb[:, 2 * GW:3 * GW]
            to2 = tgb[:, 3 * GW:4 * GW]
            nc.vector.tensor_tensor(out=ub, in0=si, in1=tg_, op=OP.mult)
            nc.vector.tensor_tensor(out=vb, in0=sf, in1=ct, op=OP.mult)
            nc.vector.tensor_tensor(out=ct, in0=ub, in1=vb, op=OP.add)
            nc.scalar.activation(tcb, ct, AF.Tanh)
            nc.vector.tensor_scalar(out=sob, in0=to2, scalar1=0.5, scalar2=0.5,
                                    op0=OP.mult, op1=OP.add)
            nc.vector.tensor_tensor(out=htile, in0=sob, in1=tcb, op=OP.mult)

        def heads_mm(t, g):
            hps = psl0.tile([4, GW], dt.float32, tag=f"l0g{g}", name="hps")
            nc.tensor.matmul(hps, lhsT=HD[:, 0:4], rhs=ht1[g][:, :],
                             start=True, stop=True)
            ring = ms8[g][(t // 8) % 2]
            nc.vector.tensor_copy(ring[:, (t % 8) * GW:(t % 8 + 1) * GW], hps)
            if t % 8 == 7 or t == TS - 1:
                k0 = t - (t % 8)
                nw = t - k0 + 1
                nc.sync.dma_start(
                    out=musig_d[:, k0:t + 1, g * GW:(g + 1) * GW],
                    in_=ring[:, 0:nw * GW].rearrange("h (s b) -> h s b", b=GW))

        def l0_tail(g):
            gate_acts(l0ps[g], tg0[g])
            cell_update(tg0[g], c0t[g], ua0[g], vb0[g], tc0[g], so0[g], ht0[g])

        def l1_tail(g):
            gate_acts(l1ps[g], tg1[g])
            cell_update(tg1[g], c1t[g], ua1[g], vb1[g], tc1[g], so1[g], ht1[g])

        l0ps, l1ps = {}, {}

        def alloc_l0(g):
            l0ps[g] = psl0.tile([128, 4 * GW], dt.float32, tag=f"l0g{g}",
                                name="l0ps")

        def alloc_l1(g):
            l1ps[g] = psl1.tile([128, 4 * GW], dt.float32, tag=f"l1g{g}",
                                name="l1ps")

        # group 1 runs a half-step behind group 0
        for t in range(TS):
            # phase 1 (PE): g0 layer0 of t; g1 layer1 of t-1; g0 heads of t-1
            alloc_l0(0)
            l0_matmuls(l0ps[0], 0, t * HALF)
            if t > 0:
                alloc_l1(1)
                l1_matmuls(l1ps[1], 1, (t - 1) * HALF + GW)
                heads_mm(t - 1, 0)
            # phase 2 (tails)
            l0_tail(0)
            if t > 0:
                l1_tail(1)
            # phase 3 (PE): g0 layer1 of t; g1 layer0 of t; g1 heads of t-1
            alloc_l1(0)
            l1_matmuls(l1ps[0], 0, t * HALF)
            alloc_l0(1)
            l0_matmuls(l0ps[1], 1, t * HALF + GW)
            if t > 0:
                heads_mm(t - 1, 1)
            # phase 4 (tails)
            l1_tail(0)
            l0_tail(1)
        alloc_l1(1)
        l1_matmuls(l1ps[1], 1, (TS - 1) * HALF + GW)
        heads_mm(TS - 1, 0)
        l1_tail(1)
        heads_mm(TS - 1, 1)

        # ---- post-pass ----------------------------------------------------
        c_sigb = persist.tile([128, 1], dt.float32, tag="c_sigb")
        nc.vector.memset(c_sigb, sigma_b)
        c_neg1 = persist.tile([128, 1], dt.float32, tag="c_neg1")
        nc.vector.memset(c_neg1, -1.0)

        for c in range(NCHUNK):
            b_, n0 = c // 2, (c % 2) * 128
            fc, mv, sd, isd = full_c[c], mv_c[c], stdev_c[c], istd_c[c]

            mu_tf = work.tile([128, 128], dt.bfloat16, tag="mu_tf")
            sg_tf = work.tile([128, 128], dt.bfloat16, tag="sg_tf")
            nc.sync.dma_start_transpose(
                out=mu_tf,
                in_=musig_d[0 + 2 * (c // 4), :, (c % 4) * 128:(c % 4 + 1) * 128])
            nc.sync.dma_start_transpose(
                out=sg_tf,
                in_=musig_d[1 + 2 * (c // 4), :, (c % 4) * 128:(c % 4 + 1) * 128])
            mu_t = mu_tf[:, 0:TS]
            sg_t = sg_tf[:, 0:TS]

            eps_c = work.tile([128, TS], dt.float32, tag="eps_c")
            nc.sync.dma_start(out=eps_c,
                              in_=epsin[b_, :, n0:n0 + 128].rearrange("t n -> n t"))
            mk = work.tile([128, TS], dt.float32, tag="mk")
            nc.sync.dma_start(out=mk[:, 0:LIN - 1],
                              in_=hmask[b_, 1:LIN, n0:n0 + 128].rearrange("t n -> n t"))
            nc.sync.dma_start(out=mk[:, LIN - 1:TS],
                              in_=fmask[b_, :, n0:n0 + 128].rearrange("t n -> n t"))

            # sigma = softplus(sg + sigma_b) + 1e-6
            ab_ = work.tile([128, TS], dt.float32, tag="ab_")
            nc.scalar.activation(ab_, sg_t, AF.Abs, bias=c_sigb)
            ex_ = work.tile([128, TS], dt.float32, tag="ex_")
            nc.scalar.activation(ex_, ab_, AF.Exp, scale=c_neg1)
            ln_ = work.tile([128, TS], dt.float32, tag="ln_")
            nc.scalar.activation(ln_, ex_, AF.Ln, bias=1.0)
            rl_ = work.tile([128, TS], dt.float32, tag="rl_")
            nc.vector.tensor_scalar(out=rl_, in0=sg_t, scalar1=sigma_b,
                                    scalar2=0.0, op0=OP.add, op1=OP.max)
            sig = work.tile([128, TS], dt.float32, tag="sig")
            nc.vector.scalar_tensor_tensor(out=sig, in0=ln_, scalar=1e-6, in1=rl_,
                                           op0=OP.add, op1=OP.add)

            # preds = ((mu+mu_b) + sigma*eps)*stdev + means, masked
            m1 = work.tile([128, TS], dt.float32, tag="m1")
            nc.vector.tensor_tensor(out=m1, in0=sig, in1=eps_c, op=OP.mult)
            m2 = work.tile([128, TS], dt.float32, tag="m2")
            nc.vector.scalar_tensor_tensor(out=m2, in0=mu_t, scalar=mu_b, in1=m1,
                                           op0=OP.add, op1=OP.add)
            m3 = work.tile([128, TS], dt.float32, tag="m3")
            nc.vector.tensor_scalar(out=m3, in0=m2, scalar1=sd, scalar2=mv[:, 0:1],
                                    op0=OP.mult, op1=OP.add)
            pr = work.tile([128, TS], dt.float32, tag="pr")
            nc.vector.tensor_tensor(out=pr, in0=m3, in1=mk, op=OP.mult)

            rr = work.tile([128, TS], dt.float32, tag="rr")
            nc.vector.tensor_tensor(out=rr, in0=fc[:, 1:T], in1=mk, op=OP.mult)

            u1 = work.tile([128, TS], dt.float32, tag="u1")
            nc.vector.tensor_scalar(out=u1, in0=mu_t, scalar1=mu_b, scalar2=None,
                                    op0=OP.add)
            u2 = work.tile([128, TS], dt.float32, tag="u2")
            nc.vector.tensor_scalar(out=u2, in0=u1, scalar1=sd, scalar2=mv[:, 0:1],
                                    op0=OP.mult, op1=OP.add)

            v1 = work.tile([128, TS], dt.float32, tag="v1")
            nc.vector.tensor_scalar(out=v1, in0=sig, scalar1=sd, scalar2=mv[:, 0:1],
                                    op0=OP.mult, op1=OP.add)

            for src_t, odram in ((pr, o_preds), (rr, o_reals), (u2, o_mus),
                                 (v1, o_sigs), (mk, o_mask)):
                tps = psl0.tile([TS, 128], dt.float32, tag="l0g0", name="tps")
                nc.tensor.transpose(tps, src_t, IDF)
                osb = work.tile([TS, 128], dt.float32, tag="osb", bufs=4)
                nc.vector.tensor_copy(osb, tps)
                nc.sync.dma_start(out=odram[b_, :, n0:n0 + 128], in_=osb)

    nc.finalize()
    return nc


def kernel(**inputs):
    import os
    from concourse.bass_utils import run_bass_kernel_spmd

    f32 = np.float32
    packs = _pack_weights(inputs)

    key = "nc"
    if key not in _cache:
        _cache[key] = _build(packs["mu_b"], packs["sigma_b"])
    nc = _cache[key]

    hist = np.ascontiguousarray(np.asarray(inputs["history_data"], f32)[..., 0])
    fut = np.ascontiguousarray(np.asarray(inputs["future_data"], f32)[..., 0])
    hm = np.ascontiguousarray(np.asarray(inputs["history_mask"], f32))
    fm = np.ascontiguousarray(np.asarray(inputs["future_mask"], f32))
    eps = np.ascontiguousarray(np.asarray(inputs["eps"], f32)[..., 0])

    in_maps = []
    for c in range(NCORES):
        b0, b1 = c * BL, (c + 1) * BL
        m = {
            "hist": hist[b0:b1], "fut": fut[b0:b1],
            "hmask": hm[b0:b1], "fmask": fm[b0:b1], "epsin": eps[b0:b1],
        }
        for k in ("WH0", "WX0", "WI1", "WH1", "B1T", "HD", "ID", "ONESBIG",
                  "IDF"):
            m[k] = packs[k]
        in_maps.append(m)

    kres = run_bass_kernel_spmd(nc, in_maps, list(range(NCORES)),
                                trace=bool(os.environ.get("KERNEL_TRACE")))
    _cache["last"] = kres
    res = kres.results

    def gather(name):
        full = np.concatenate([res[c][name] for c in range(NCORES)], axis=0)
        return full.reshape(B, TS, N, 1).astype(f32)

    return (gather("preds"), gather("reals"), gather("musv"),
            gather("sigmasv"), gather("maskv"))


# revision 27
# speedup vs baseline: 1.2591x; 1.1909x over previous
"""DeepAR Trainium2 Bass kernel.

Strategy (hardcoded from spec nn_DeepAR_90374701843258):
  B=32, LIN=96, LOUT=24, N=256, E=32, H=64, T-1=119 steps, 8 cores.
  Data-parallel over B: 4 batch rows per core -> per-core batch BN=1024.

  Layout: "folded" tiles: partition p<64 = H-unit p of batch half 0
  (bn 0:512), p>=64 = H-unit p-64 of half 1 (bn 512:1024). The free dim
  is the within-half batch column. The 1024 batch is further split into
  two independent GROUPS (cols 0:256, 256:512 of each half) that pipeline
  against each other across engines.

  Algebra:
   - embedding + layer0 input proj collapse to rank-1: pre0 = x*w_eff + b_eff
     (w_eff = Wih0 @ embed_W); x and ones live in a pre-staged xall buffer
     (partitions 0/1/64/65, one column block per step) and enter the gate
     matmul as extra contraction rows.
   - block-diagonal stationary diag(Wx^T, Wx^T) [128,128] lets one matmul
     produce a folded gate tile (both halves) per group.
   - i,f gates: real Sigmoid on ACT; g: Tanh; o: tanh(o/2) via 0.5-prescaled
     weights, sigmoid recovered as 0.5*tanh+0.5 with a 4x-mode tensor_scalar.
   - cell update: u=si*tg, v=sf*c, c'=u+v, tc=tanh(c'), so=0.5*to2+0.5,
     h=so*tc  (all bf16 TT at 2x / TS at 4x DVE modes).
"""

import numpy as np

B, LIN, LOUT, N, E, H = 32, 96, 24, 256, 32, 64
T = LIN + LOUT
TS = T - 1            # 119
NCORES = 8
BL = B // NCORES      # 4
BN = BL * N           # 1024
HALF = 512
GW = 512              # single stream: full folded width
NCHUNK = BN // 128    # 8

_cache = {}


def _pack_weights(inp):
    """Host-side weight prep (tiny arrays)."""
    import ml_dtypes
    bf16 = ml_dtypes.bfloat16
    f32 = np.float32

    Wih0, Whh0 = inp["Wih0"].astype(f32), inp["Whh0"].astype(f32)
    Wih1, Whh1 = inp["Wih1"].astype(f32), inp["Whh1"].astype(f32)
    w_eff = (Wih0 @ inp["embed_W"].astype(f32))[:, 0]
    b_eff = Wih0 @ inp["embed_b"].astype(f32) + inp["bih0"] + inp["bhh0"]
    b1 = (inp["bih1"] + inp["bhh1"]).astype(f32)

    sc = np.ones(4 * H, f32)
    sc[3 * H:] = 0.5       # o-gate only

    def blockdiag(Wm):
        out = np.zeros((128, 4 * 128), f32)
        for X in range(4):
            wt = (Wm[X * H:(X + 1) * H].T * sc[X * H:(X + 1) * H][None, :])
            out[0:64, X * 128:X * 128 + 64] = wt
            out[64:128, X * 128 + 64:(X + 1) * 128] = wt
        return out

    WH0 = blockdiag(Whh0)
    WI1 = blockdiag(Wih1)
    WH1 = blockdiag(Whh1)

    WX0 = np.zeros((128, 4 * 128), f32)
    for X in range(4):
        we = w_eff[X * H:(X + 1) * H] * sc[X * H:(X + 1) * H]
        be = b_eff[X * H:(X + 1) * H] * sc[X * H:(X + 1) * H]
        WX0[0, X * 128 + 64:(X + 1) * 128] = we   # x half1 -> out parts 64:128
        WX0[1, X * 128 + 64:(X + 1) * 128] = be
        WX0[64, X * 128:X * 128 + 64] = we        # x half0 -> out parts 0:64
        WX0[65, X * 128:X * 128 + 64] = be

    B1F = np.zeros((128, 4), f32)
    for X in range(4):
        bb = b1[X * H:(X + 1) * H] * sc[X * H:(X + 1) * H]
        B1F[0:64, X] = bb
        B1F[64:128, X] = bb

    HD = np.zeros((128, 4), f32)
    HD[0:64, 0] = inp["mu_W"].astype(f32)[0]
    HD[0:64, 1] = inp["sigma_W"].astype(f32)[0]
    HD[64:128, 2] = inp["mu_W"].astype(f32)[0]
    HD[64:128, 3] = inp["sigma_W"].astype(f32)[0]

    return {
        "WH0": WH0.astype(bf16), "WX0": WX0.astype(bf16),
        "WI1": WI1.astype(bf16), "WH1": WH1.astype(bf16),
        "B1F": B1F, "HD": HD.astype(bf16),
        "ID": np.eye(128, dtype=f32).astype(bf16),
        "ONESBIG": np.ones((1, TS * HALF), f32).astype(bf16),
        "IDF": np.eye(128, dtype=f32),
        "mu_b": float(inp["mu_b"][0]), "sigma_b": float(inp["sigma_b"][0]),
    }


def _build(mu_b, sigma_b):
    """Build the per-core bass program (SPMD: identical on all cores)."""
    from contextlib import ExitStack
    import concourse.mybir as mybir
    import concourse.tile as tile
    from concourse import bacc

    dt = mybir.dt
    AF = mybir.ActivationFunctionType
    OP = mybir.AluOpType

    nc = bacc.Bacc()

    # ---- I/O ----------------------------------------------------------
    hist = nc.declare_dram_parameter("hist", [BL, LIN, N], dt.float32, isOutput=False)
    fut = nc.declare_dram_parameter("fut", [BL, LOUT, N], dt.float32, isOutput=False)
    hmask = nc.declare_dram_parameter("hmask", [BL, LIN, N], dt.float32, isOutput=False)
    fmask = nc.declare_dram_parameter("fmask", [BL, LOUT, N], dt.float32, isOutput=False)
    epsin = nc.declare_dram_parameter("epsin", [BL, TS, N], dt.float32, isOutput=False)
    wWH0 = nc.declare_dram_parameter("WH0", [128, 512], dt.bfloat16, isOutput=False)
    wWX0 = nc.declare_dram_parameter("WX0", [128, 512], dt.bfloat16, isOutput=False)
    wWI1 = nc.declare_dram_parameter("WI1", [128, 512], dt.bfloat16, isOutput=False)
    wWH1 = nc.declare_dram_parameter("WH1", [128, 512], dt.bfloat16, isOutput=False)
    wB1F = nc.declare_dram_parameter("B1F", [128, 4], dt.float32, isOutput=False)
    wHD = nc.declare_dram_parameter("HD", [128, 4], dt.bfloat16, isOutput=False)
    wID = nc.declare_dram_parameter("ID", [128, 128], dt.bfloat16, isOutput=False)
    wONB = nc.declare_dram_parameter("ONESBIG", [1, TS * HALF], dt.bfloat16, isOutput=False)
    wIDF = nc.declare_dram_parameter("IDF", [128, 128], dt.float32, isOutput=False)

    o_preds = nc.declare_dram_parameter("preds", [BL, TS, N], dt.float32, isOutput=True)
    o_reals = nc.declare_dram_parameter("reals", [BL, TS, N], dt.float32, isOutput=True)
    o_mus = nc.declare_dram_parameter("musv", [BL, TS, N], dt.float32, isOutput=True)
    o_sigs = nc.declare_dram_parameter("sigmasv", [BL, TS, N], dt.float32, isOutput=True)
    o_mask = nc.declare_dram_parameter("maskv", [BL, TS, N], dt.float32, isOutput=True)

    musig_d = nc.dram_tensor("musig", [4, 128, HALF], dt.bfloat16)
    xs_d = nc.dram_tensor("xsd", [TS, BN], dt.bfloat16)

    with ExitStack() as ctx:
        tc = ctx.enter_context(tile.TileContext(nc))
        persist = ctx.enter_context(tc.tile_pool(name="persist", bufs=1))
        work = ctx.enter_context(tc.tile_pool(name="work", bufs=3))
        psl0 = ctx.enter_context(tc.tile_pool(name="psl0", bufs=1, space="PSUM"))
        psl1 = ctx.enter_context(tc.tile_pool(name="psl1", bufs=1, space="PSUM"))

        # ---- weights ------------------------------------------------------
        WH0 = persist.tile([128, 512], dt.bfloat16, tag="WH0")
        WX0 = persist.tile([128, 512], dt.bfloat16, tag="WX0")
        WI1 = persist.tile([128, 512], dt.bfloat16, tag="WI1")
        WH1 = persist.tile([128, 512], dt.bfloat16, tag="WH1")
        HD = persist.tile([128, 4], dt.bfloat16, tag="HD")
        ID = persist.tile([128, 128], dt.bfloat16, tag="ID")
        IDF = persist.tile([128, 128], dt.float32, tag="IDF")
        B1F = persist.tile([128, 4], dt.float32, tag="B1F")
        for t_, d_ in [(WH0, wWH0), (WX0, wWX0), (WI1, wWI1), (WH1, wWH1),
                       (B1F, wB1F), (HD, wHD), (ID, wID), (IDF, wIDF)]:
            nc.sync.dma_start(out=t_[:], in_=d_[:])

        # ---- state tiles (ht0/ht1 ping-pong to decouple WARs) -------------
        def gtile(nm, shape, dtype=dt.bfloat16):
            return [persist.tile(shape, dtype, tag=f"{nm}{g}", name=f"{nm}{g}")
                    for g in range(2)]

        ht0 = gtile("ht0", [128, GW])
        ht1 = gtile("ht1", [128, GW])
        c0t = persist.tile([128, GW], dt.bfloat16, tag="c0t")
        c1t = persist.tile([128, GW], dt.bfloat16, tag="c1t")
        for t_ in (ht0[0], ht0[1], ht1[0], ht1[1], c0t, c1t):
            nc.vector.memset(t_, 0.0)
        tg0 = persist.tile([128, 4 * GW], dt.bfloat16, tag="tg0")
        tg1 = persist.tile([128, 4 * GW], dt.bfloat16, tag="tg1")
        ua0 = persist.tile([128, GW], dt.bfloat16, tag="ua0")
        vb0 = persist.tile([128, GW], dt.bfloat16, tag="vb0")
        ua1 = persist.tile([128, GW], dt.bfloat16, tag="ua1")
        vb1 = persist.tile([128, GW], dt.bfloat16, tag="vb1")
        tc0 = persist.tile([128, GW], dt.bfloat16, tag="tc0")
        tc1 = persist.tile([128, GW], dt.bfloat16, tag="tc1")
        so0 = persist.tile([128, GW], dt.bfloat16, tag="so0")
        so1 = persist.tile([128, GW], dt.bfloat16, tag="so1")
        ms8 = [persist.tile([4, 8 * GW], dt.bfloat16, tag=f"ms8{i}",
                            name=f"ms8{i}") for i in range(2)]

        xall = persist.tile([128, TS * HALF], dt.bfloat16, tag="xall")
        nc.vector.memset(xall, 0.0)
        for p_ in (1, 32, 65):
            nc.sync.dma_start(out=xall[p_:p_ + 1, :], in_=wONB[:])

        xt = persist.tile([TS, BN], dt.bfloat16, tag="xt")

        full_c, mv_c, stdev_c, istd_c = [], [], [], []

        # ---- pre-pass: stats, normalize, transpose x ----------------------
        for c in range(NCHUNK):
            b_, n0 = c // 2, (c % 2) * 128
            raw = work.tile([T, 128], dt.float32, tag="raw")
            nc.sync.dma_start(out=raw[0:LIN, :], in_=hist[b_, :, n0:n0 + 128])
            nc.sync.dma_start(out=raw[LIN:T, :], in_=fut[b_, :, n0:n0 + 128])
            fpt = psl1.tile([128, T], dt.float32, tag="l1", name="fpt")
            nc.tensor.transpose(fpt, raw, IDF[0:T, 0:T])
            fc = persist.tile([128, T], dt.float32, tag=f"full{c}", name=f"full{c}")
            nc.vector.tensor_copy(fc, fpt)

            st6 = work.tile([128, 6], dt.float32, tag="st6")
            mv = persist.tile([128, 2], dt.float32, tag=f"mv{c}", name=f"mv{c}")
            nc.vector.bn_stats(out=st6, in_=fc[:, 0:LIN])
            nc.vector.bn_aggr(out=mv, in_=st6)
            veps = work.tile([128, 1], dt.float32, tag="veps")
            nc.vector.tensor_scalar(out=veps, in0=mv[:, 1:2], scalar1=1e-5,
                                    scalar2=None, op0=OP.add)
            y0 = work.tile([128, 1], dt.float32, tag="y0")
            nc.scalar.activation(y0, veps, AF.Sqrt)
            r0 = work.tile([128, 1], dt.float32, tag="r0")
            nc.vector.reciprocal(r0, y0)
            yy = work.tile([128, 1], dt.float32, tag="yy")
            nc.vector.tensor_tensor(out=yy, in0=y0, in1=y0, op=OP.mult)
            e_ = work.tile([128, 1], dt.float32, tag="e_")
            nc.vector.tensor_tensor(out=e_, in0=veps, in1=yy, op=OP.subtract)
            d_ = work.tile([128, 1], dt.float32, tag="d_")
            nc.vector.scalar_tensor_tensor(out=d_, in0=e_, scalar=0.5, in1=r0,
                                           op0=OP.mult, op1=OP.mult)
            sd = persist.tile([128, 1], dt.float32, tag=f"sd{c}", name=f"sd{c}")
            nc.vector.tensor_tensor(out=sd, in0=y0, in1=d_, op=OP.add)
            isd = persist.tile([128, 1], dt.float32, tag=f"isd{c}", name=f"isd{c}")
            nc.vector.reciprocal(isd, sd)
            full_c.append(fc); mv_c.append(mv); stdev_c.append(sd); istd_c.append(isd)

            xn = work.tile([128, TS], dt.bfloat16, tag="xn")
            nc.vector.tensor_scalar(out=xn, in0=fc[:, 0:TS], scalar1=mv[:, 0:1],
                                    scalar2=isd, op0=OP.subtract, op1=OP.mult)
            pt = psl0.tile([TS, 128], dt.bfloat16, tag="l0", name="pt")
            nc.tensor.transpose(pt, xn, ID)
            xtcol = (1 - c // 4) * HALF + (c % 4) * 128
            nc.vector.tensor_copy(xt[:, xtcol:xtcol + 128], pt)

        # stage xt -> DRAM -> xall partitions 0 / 64
        nc.sync.dma_start(out=xs_d[:], in_=xt[:])
        nc.sync.dma_start(
            out=xall[0:1, :].rearrange("p (t b) -> p t b", b=HALF),
            in_=xs_d[None, :, 0:HALF])
        nc.sync.dma_start(
            out=xall[64:65, :].rearrange("p (t b) -> p t b", b=HALF),
            in_=xs_d[None, :, HALF:BN])

        # ---- main loop: L1 trails L0 by one step; heads by two -----------
        GS = [slice(X * 128, (X + 1) * 128) for X in range(4)]

        def l0_matmuls(ps, t):
            ts_ = t * HALF
            h = ht0[(t - 1) % 2]
            for X in range(4):
                nc.tensor.matmul(ps[:, X * GW:(X + 1) * GW], lhsT=WH0[:, GS[X]],
                                 rhs=h[:, :], start=True, stop=False)
                nc.tensor.matmul(ps[:, X * GW:(X + 1) * GW], lhsT=WX0[0:66, GS[X]],
                                 rhs=xall[0:66, ts_:ts_ + GW],
                                 start=False, stop=True)

        def l1_matmuls(ps, t):
            h0_ = ht0[t % 2]
            h1_ = ht1[(t - 1) % 2]
            for X in range(4):
                nc.tensor.matmul(ps[:, X * GW:(X + 1) * GW], lhsT=WI1[:, GS[X]],
                                 rhs=h0_[:, :], start=True, stop=False)
                nc.tensor.matmul(ps[:, X * GW:(X + 1) * GW], lhsT=WH1[:, GS[X]],
                                 rhs=h1_[:, :], start=False, stop=True)

        def cell_update(tgb, ct, ub, vb, tcb, sob, htile):
            si = tgb[:, 0:GW]
            sf = tgb[:, GW:2 * GW]
            tg_ = tgb[:, 2 * GW:3 * GW]
            to2 = tgb[:, 3 * GW:4 * GW]
            nc.vector.tensor_tensor(out=ub, in0=si, in1=tg_, op=OP.mult)
            nc.vector.tensor_tensor(out=vb, in0=sf, in1=ct, op=OP.mult)
            nc.vector.tensor_tensor(out=ct, in0=ub, in1=vb, op=OP.add)
            nc.scalar.activation(tcb, ct, AF.Tanh)
            nc.vector.tensor_scalar(out=sob, in0=to2, scalar1=0.5, scalar2=0.5,
                                    op0=OP.mult, op1=OP.add)
            nc.vector.tensor_tensor(out=htile, in0=sob, in1=tcb, op=OP.mult)

        def heads_mm(t):
            hps = psl0.tile([4, GW], dt.float32, tag="l0", name="hps")
            nc.tensor.matmul(hps, lhsT=HD[:, 0:4], rhs=ht1[t % 2][:, :],
                             start=True, stop=True)
            ring = ms8[(t // 8) % 2]
            nc.vector.tensor_copy(ring[:, (t % 8) * GW:(t % 8 + 1) * GW], hps)
            if t % 8 == 7 or t == TS - 1:
                k0 = t - (t % 8)
                nw = t - k0 + 1
                nc.sync.dma_start(
                    out=musig_d[:, k0:t + 1, :],
                    in_=ring[:, 0:nw * GW].rearrange("h (s b) -> h s b", b=GW))

        def l0_tail(t):
            nc.scalar.activation(tg0[:, 0:2 * GW], l0ps[0][:, 0:2 * GW],
                                 AF.Sigmoid)
            nc.scalar.activation(tg0[:, 2 * GW:4 * GW],
                                 l0ps[0][:, 2 * GW:4 * GW], AF.Tanh)
            cell_update(tg0, c0t, ua0, vb0, tc0, so0, ht0[t % 2])

        def l1_tail(t):
            for X, fn in ((0, AF.Sigmoid), (2, AF.Tanh), (1, AF.Sigmoid),
                          (3, AF.Tanh)):
                nc.scalar.activation(tg1[:, X * GW:(X + 1) * GW],
                                     l1ps[0][:, X * GW:(X + 1) * GW], fn,
                                     bias=B1F[:, X:X + 1])
            cell_update(tg1, c1t, ua1, vb1, tc1, so1, ht1[t % 2])

        l0ps, l1ps = [None], [None]
        for t in range(TS):
            l0ps[0] = psl0.tile([128, 4 * GW], dt.float32, tag="l0", name="l0ps")
            l0_matmuls(l0ps[0], t)
            if t >= 2:
                heads_mm(t - 2)
            l0_tail(t)
            if t >= 1:
                l1ps[0] = psl1.tile([128, 4 * GW], dt.float32, tag="l1",
                                    name="l1ps")
                l1_matmuls(l1ps[0], t - 1)
                l1_tail(t - 1)
        l1ps[0] = psl1.tile([128, 4 * GW], dt.float32, tag="l1", name="l1ps")
        l1_matmuls(l1ps[0], TS - 1)
        l1_tail(TS - 1)
        heads_mm(TS - 2)
        heads_mm(TS - 1)

        # ---- post-pass ----------------------------------------------------
        c_sigb = persist.tile([128, 1], dt.float32, tag="c_sigb")
        nc.vector.memset(c_sigb, sigma_b)
        c_neg1 = persist.tile([128, 1], dt.float32, tag="c_neg1")
        nc.vector.memset(c_neg1, -1.0)

        for c in range(NCHUNK):
            b_, n0 = c // 2, (c % 2) * 128
            fc, mv, sd, isd = full_c[c], mv_c[c], stdev_c[c], istd_c[c]

            mu_tf = work.tile([128, 128], dt.bfloat16, tag="mu_tf")
            sg_tf = work.tile([128, 128], dt.bfloat16, tag="sg_tf")
            nc.sync.dma_start_transpose(
                out=mu_tf,
                in_=musig_d[0 + 2 * (c // 4), :, (c % 4) * 128:(c % 4 + 1) * 128])
            nc.sync.dma_start_transpose(
                out=sg_tf,
                in_=musig_d[1 + 2 * (c // 4), :, (c % 4) * 128:(c % 4 + 1) * 128])
            mu_t = mu_tf[:, 0:TS]
            sg_t = sg_tf[:, 0:TS]

            eps_c = work.tile([128, TS], dt.float32, tag="eps_c")
            nc.sync.dma_start(out=eps_c,
                              in_=epsin[b_, :, n0:n0 + 128].rearrange("t n -> n t"))
            mk = work.tile([128, TS], dt.float32, tag="mk")
            nc.sync.dma_start(out=mk[:, 0:LIN - 1],
                              in_=hmask[b_, 1:LIN, n0:n0 + 128].rearrange("t n -> n t"))
            nc.sync.dma_start(out=mk[:, LIN - 1:TS],
                              in_=fmask[b_, :, n0:n0 + 128].rearrange("t n -> n t"))

            # sigma = softplus(sg + sigma_b) + 1e-6
            ab_ = work.tile([128, TS], dt.float32, tag="ab_")
            nc.scalar.activation(ab_, sg_t, AF.Abs, bias=c_sigb)
            ex_ = work.tile([128, TS], dt.float32, tag="ex_")
            nc.scalar.activation(ex_, ab_, AF.Exp, scale=c_neg1)
            ln_ = work.tile([128, TS], dt.float32, tag="ln_")
            nc.scalar.activation(ln_, ex_, AF.Ln, bias=1.0)
            rl_ = work.tile([128, TS], dt.float32, tag="rl_")
            nc.vector.tensor_scalar(out=rl_, in0=sg_t, scalar1=sigma_b,
                                    scalar2=0.0, op0=OP.add, op1=OP.max)
            sig = work.tile([128, TS], dt.float32, tag="sig")
            nc.vector.scalar_tensor_tensor(out=sig, in0=ln_, scalar=1e-6, in1=rl_,
                                           op0=OP.add, op1=OP.add)

            # preds = ((mu+mu_b) + sigma*eps)*stdev + means, masked
            m1 = work.tile([128, TS], dt.float32, tag="m1")
            nc.vector.tensor_tensor(out=m1, in0=sig, in1=eps_c, op=OP.mult)
            m2 = work.tile([128, TS], dt.float32, tag="m2")
            nc.vector.scalar_tensor_tensor(out=m2, in0=mu_t, scalar=mu_b, in1=m1,
                                           op0=OP.add, op1=OP.add)
            m3 = work.tile([128, TS], dt.float32, tag="m3")
            nc.vector.tensor_scalar(out=m3, in0=m2, scalar1=sd, scalar2=mv[:, 0:1],
                                    op0=OP.mult, op1=OP.add)
            pr = work.tile([128, TS], dt.float32, tag="pr")
            nc.vector.tensor_tensor(out=pr, in0=m3, in1=mk, op=OP.mult)

            rr = work.tile([128, TS], dt.float32, tag="rr")
            nc.vector.tensor_tensor(out=rr, in0=fc[:, 1:T], in1=mk, op=OP.mult)

            u1 = work.tile([128, TS], dt.float32, tag="u1")
            nc.vector.tensor_scalar(out=u1, in0=mu_t, scalar1=mu_b, scalar2=None,
                                    op0=OP.add)
            u2 = work.tile([128, TS], dt.float32, tag="u2")
            nc.vector.tensor_scalar(out=u2, in0=u1, scalar1=sd, scalar2=mv[:, 0:1],
                                    op0=OP.mult, op1=OP.add)

            v1 = work.tile([128, TS], dt.float32, tag="v1")
            nc.vector.tensor_scalar(out=v1, in0=sig, scalar1=sd, scalar2=mv[:, 0:1],
                                    op0=OP.mult, op1=OP.add)

            for src_t, odram in ((pr, o_preds), (rr, o_reals), (u2, o_mus),
                                 (v1, o_sigs), (mk, o_mask)):
                tps = psl0.tile([TS, 128], dt.float32, tag="l0", name="tps")
                nc.tensor.transpose(tps, src_t, IDF)
                osb = work.tile([TS, 128], dt.float32, tag="osb", bufs=4)
                nc.vector.tensor_copy(osb, tps)
                nc.sync.dma_start(out=odram[b_, :, n0:n0 + 128], in_=osb)

    nc.finalize()
    return nc


def kernel(**inputs):
    import os
    from concourse.bass_utils import run_bass_kernel_spmd

    f32 = np.float32
    packs = _pack_weights(inputs)

    key = "nc"
    if key not in _cache:
        _cache[key] = _build(packs["mu_b"], packs["sigma_b"])
    nc = _cache[key]

    hist = np.ascontiguousarray(np.asarray(inputs["history_data"], f32)[..., 0])
    fut = np.ascontiguousarray(np.asarray(inputs["future_data"], f32)[..., 0])
    hm = np.ascontiguousarray(np.asarray(inputs["history_mask"], f32))
    fm = np.ascontiguousarray(np.asarray(inputs["future_mask"], f32))
    eps = np.ascontiguousarray(np.asarray(inputs["eps"], f32)[..., 0])

    in_maps = []
    for c in range(NCORES):
        b0, b1 = c * BL, (c + 1) * BL
        m = {
            "hist": hist[b0:b1], "fut": fut[b0:b1],
            "hmask": hm[b0:b1], "fmask": fm[b0:b1], "epsin": eps[b0:b1],
        }
        for k in ("WH0", "WX0", "WI1", "WH1", "B1F", "HD", "ID", "ONESBIG",
                  "IDF"):
            m[k] = packs[k]
        in_maps.append(m)

    kres = run_bass_kernel_spmd(nc, in_maps, list(range(NCORES)),
                                trace=bool(os.environ.get("KERNEL_TRACE")))
    _cache["last"] = kres
    res = kres.results

    def gather(name):
        full = np.concatenate([res[c][name] for c in range(NCORES)], axis=0)
        return full.reshape(B, TS, N, 1).astype(f32)

    return (gather("preds"), gather("reals"), gather("musv"),
            gather("sigmasv"), gather("maskv"))
